# revision 1
# baseline (speedup 1.0000x reference)
"""Trainium2 Bass kernel for nn_Attention_73289321939579.

Gated attention block (AlphaFold-style):
  qkv = q_x @ w_qkv.T ; q /= sqrt(64)
  scores = q k^T + bias ; attn = softmax(scores, keys)
  o = (attn @ v) * sigmoid(q_x @ w_g.T + b_g)
  out = o @ w_o.T + b_o

Sharding over 8 cores: core = b*4 + qh*2 + hq
  b  = batch (2)            -> data parallel
  qh = query half (2x1024)  -> bias/q sliced, output row-sliced
  hq = head quad (2x4 heads)-> tensor parallel; partial outputs summed on host

Device layout (per core, everything transposed so contractions sit on the
SBUF partition axis):
  xT   [512, 2048]  = q_x[b].T, key-rolled so this core's queries are cols 0:1024
  eb   [2048, 1024] = exp(bias[b,0]).T  (rows key-rolled, cols query-sliced)
        -> softmax computed as exp(qk) * exp(bias); exact in fp32
  wT   [512, 1024]  = [wq.T/8 | wk.T | wv.T | wg.T] for this core's 4 heads
  woT  [256, 512]   = w_o[:, heads].T (pair-major)
  bg   [256, 1]     = 0.5*b_g[heads]   (gate via 0.5*tanh(0.5x+0.5bg)+0.5,
                       Tanh shares the ACT "exp" table set -> no table swap)

PE discipline: every matmul in the kernel is plain 128x128 mode - a tiling
mode switch drains the array and drops the HAM clock gate to half speed
(measured 723ns/matmul alternating vs 227ns mono-mode). S^T uses the k
head-PAIR tile as lhsT (K=128) against per-head q tiles zero-padded in the
other head's 64 partitions. V is augmented with a ones column so each head's
[65, 512] PSUM accumulator carries O^T in rows 0:63 and softmax denominators
in row 64. Gate and output projections work on head pairs (M=128); the
odd-head halves are relocated across partitions with small SBUF->SBUF DMAs.
Normalization: reciprocal_approx_fast + an all-ones matmul broadcast.
All matmul operands are float32r (fp22, single-pass on the PE).

DMA discipline: constants first (so the PE warmup burst and first
projections start within ~2us), bias tiles ride the ACT HWDGE queue so they
don't queue behind bulk input loads on the SP queue.
"""

import sys

for _p in ("/opt/trn_rl_repo",):
    if _p not in sys.path:
        sys.path.insert(0, _p)

import numpy as np

import concourse.bass as bass  # noqa: F401
import concourse.mybir as mybir
import concourse.tile as tile
from concourse import bacc
from concourse.bass_utils import run_bass_kernel_spmd

# ---- problem dims (hardcoded per contest contract) ----
B, Q, CQ = 2, 2048, 512
H, D = 8, 64
P = 128
QL = 1024          # queries per core
EL = 256           # e-dims per core (4 heads x 64)
HL = 4             # heads per core
CC = CQ // P       # 4 contraction chunks over channels
EC = EL // P       # 2 head-pairs
NJ = Q // P        # 16 key chunks
NI = QL // 512     # 2 query chunks of 512

F32 = mybir.dt.float32
F32R = mybir.dt.float32r
MUL = mybir.AluOpType.mult
ADD = mybir.AluOpType.add
EXP = mybir.ActivationFunctionType.Exp
TANH = mybir.ActivationFunctionType.Tanh

OFF_Q, OFF_K, OFF_V, OFF_G = 0, EL, 2 * EL, 3 * EL


def _r(ap):
    """float32r view for matmul operands (single-pass fp22 on the PE)."""
    return ap.bitcast(F32R)


def _emit(tc, xT, eb, wT, woT, bg, ones_in, zz_in, ident_in, outp):
    nc = tc.nc
    from contextlib import ExitStack

    with ExitStack() as ctx:
        const = ctx.enter_context(tc.tile_pool(name="const", bufs=1))
        biasp = ctx.enter_context(tc.tile_pool(name="biasp", bufs=3))
        esp = ctx.enter_context(tc.tile_pool(name="esp", bufs=3))
        ptp = ctx.enter_context(tc.tile_pool(name="ptp", bufs=4))
        workp = ctx.enter_context(tc.tile_pool(name="workp", bufs=2))
        odp = ctx.enter_context(tc.tile_pool(name="odp", bufs=4))
        psum = ctx.enter_context(tc.tile_pool(name="psum", bufs=2, space="PSUM"))

        # ---- small constants FIRST so compute can start immediately ----
        ones_sb = const.tile([P, P], F32R, name="ones_sb", tag="ones_sb")
        nc.sync.dma_start(ones_sb, ones_in)
        ident_sb = const.tile([P, P], F32R, name="ident_sb", tag="ident_sb")
        nc.sync.dma_start(ident_sb, ident_in)
        bg_sb = const.tile([P, EC], F32, name="bg_sb", tag="bg_sb")
        nc.sync.dma_start(bg_sb, bg.rearrange("(o p) u -> p (o u)", p=P))
        # woT pair-major: [128 partitions, pair, 512]
        woT_sb = const.tile([P, EC, CQ], F32R, name="woT_sb", tag="woT_sb")
        nc.sync.dma_start(woT_sb, woT.rearrange("(o p) c -> p o c", p=P))
        rec_sbs = []
        for ri in range(2):
            rcb = const.tile([P, 512], F32R, name=f"rec_sb{ri}", tag=f"rec_sb{ri}")
            nc.sync.dma_start(rcb[0:64, :], zz_in[:, 0:512])
            nc.sync.dma_start(rcb[64:128, :], zz_in[:, 0:512])
            rec_sbs.append(rcb)

        # ---- bulk inputs, chunked, in consumption order ----
        wT_sb = const.tile([P, CC, 4 * EL], F32R, name="wT_sb", tag="wT_sb")
        wTr = wT.rearrange("(o p) n -> p o n", p=P)
        for c in range(CC):
            nc.sync.dma_start(wT_sb[:, c, :], wTr[:, c, :])
        xT_sb = const.tile([P, CC, Q], F32R, name="xT_sb", tag="xT_sb")
        xTr = xT.rearrange("(o p) q -> p o q", p=P)
        for j4 in range(Q // 512):
            for c in range(CC):
                nc.sync.dma_start(
                    xT_sb[:, c, j4 * 512 : (j4 + 1) * 512],
                    xTr[:, c, j4 * 512 : (j4 + 1) * 512],
                )

        # ---- resident intermediates ----
        kT_sb = const.tile([P, EC, Q], F32R, name="kT_sb", tag="kT_sb")
        # per-head q, zero-padded in the other head's 64 partitions:
        # head h occupies partitions (h%2)*64 : (h%2)*64+64, rest is zero
        qTp_sb = const.tile([P, HL, QL], F32R, name="qTp_sb", tag="qTp_sb")
        # gate in pair layout + relocated odd-head halves
        gp_sb = const.tile([P, EC, QL], F32, name="gp_sb", tag="gp_sb")
        # og in pair layout for the output projection (odd heads staged+moved)
        og_sb = const.tile([P, EC, QL], F32R, name="og_sb", tag="og_sb")
        # V augmented with a ones column: [j, head, 65]
        v_sb = const.tile([P, NJ, HL, D + 1], F32R, name="v_sb", tag="v_sb")

        # zero the pad halves of qTp: heads 0,2 pad partitions 64:128,
        # heads 1,3 pad partitions 0:64
        zz = zz_in.rearrange("p (a q) -> p a q", a=2)
        nc.sync.dma_start(qTp_sb[64:128, 0::2, :], zz)
        nc.sync.dma_start(qTp_sb[0:64, 1::2, :], zz)

        # V_aug ones column (single strided copy from the ones tile)
        nc.vector.tensor_copy(
            out=v_sb[:, :, :, D],
            in_=ones_sb[:, 0:64].rearrange("p (a b) -> p a b", a=NJ),
        )

        # ---- phase 0: PE warmup burst (HAM needs ~3.4us of activity) ----
        warm_ps = psum.tile([P, 2, 512], F32, tag="s", name="warm_ps")
        for wi in range(120):
            nc.tensor.matmul(
                warm_ps[:, 0, 0:P],
                _r(ones_sb),
                _r(ones_sb),
                start=(wi == 0),
                stop=(wi == 119),
            )
        warm_sb = workp.tile([P, P], F32, name="warm_sb", tag="recf")
        nc.vector.tensor_copy(out=warm_sb[:, 0:P], in_=warm_ps[:, 0, 0:P])

        # ---- phase 1: projections (all 128x128) ----
        # gate pairs: sigmoid(x wg^T + bg) = 0.5*tanh(0.5 x wg^T + 0.5 bg)+0.5
        for ec in range(EC):
            for ic in range(NI):
                ps_g = psum.tile([P, 512], F32, tag="s", name="ps_g")
                for c in range(CC):
                    nc.tensor.matmul(
                        ps_g,
                        _r(wT_sb[:, c, OFF_G + ec * P : OFF_G + (ec + 1) * P]),
                        _r(xT_sb[:, c, ic * 512 : (ic + 1) * 512]),
                        start=(c == 0),
                        stop=(c == CC - 1),
                    )
                nc.scalar.activation(
                    gp_sb[:, ec, ic * 512 : (ic + 1) * 512],
                    ps_g,
                    TANH,
                    bias=bg_sb[:, ec : ec + 1],
                    scale=0.5,
                )
        nc.vector.tensor_scalar(gp_sb, gp_sb, 0.5, 0.5, MUL, ADD)
        # kT pair-layout over all 2048 keys
        for ec in range(EC):
            for j4 in range(Q // 512):
                ps_k = psum.tile([P, 512], F32, tag="s", name="ps_k")
                for c in range(CC):
                    nc.tensor.matmul(
                        ps_k,
                        _r(wT_sb[:, c, OFF_K + ec * P : OFF_K + (ec + 1) * P]),
                        _r(xT_sb[:, c, j4 * 512 : (j4 + 1) * 512]),
                        start=(c == 0),
                        stop=(c == CC - 1),
                    )
                nc.vector.tensor_copy(
                    out=kT_sb[:, ec, j4 * 512 : (j4 + 1) * 512], in_=ps_k
                )
        # q pair-projected, split into the zero-padded per-head layout
        # (psum rows 0:64 = even head, 64:128 = odd head -> same partitions)
        for ec in range(EC):
            for ic in range(NI):
                ps_q = psum.tile([P, 512], F32, tag="s", name="ps_q")
                for c in range(CC):
                    nc.tensor.matmul(
                        ps_q,
                        _r(wT_sb[:, c, OFF_Q + ec * P : OFF_Q + (ec + 1) * P]),
                        _r(xT_sb[:, c, ic * 512 : (ic + 1) * 512]),
                        start=(c == 0),
                        stop=(c == CC - 1),
                    )
                sl = slice(ic * 512, (ic + 1) * 512)
                nc.any.tensor_copy(out=qTp_sb[0:64, 2 * ec, sl], in_=ps_q[0:64, :])
                nc.any.tensor_copy(
                    out=qTp_sb[64:128, 2 * ec + 1, sl], in_=ps_q[64:128, :]
                )
        # v[j, e] (keys on partitions), scattered into the aug layout
        for jc in range(NJ):
            ps_v = psum.tile([P, 512], F32, tag="s", name="ps_v")
            for c in range(CC):
                nc.tensor.matmul(
                    ps_v[:, :EL],
                    _r(xT_sb[:, c, jc * P : (jc + 1) * P]),
                    _r(wT_sb[:, c, OFF_V : OFF_V + EL]),
                    start=(c == 0),
                    stop=(c == CC - 1),
                )
            nc.any.tensor_copy(
                out=v_sb[:, jc, :, 0:D],
                in_=ps_v[:, :EL].rearrange("p (h d) -> p h d", h=HL),
            )

        # ---- phase 2: attention (128x128 throughout) ----
        # hp-blocked j-loop: only one head-pair's O accumulators live at a
        # time (2 PSUM banks), buying a 3rd S slot so the PE can run ahead
        # of the exp pipeline. The bias tile is re-fetched per pair (the ACT
        # HWDGE queue has the spare bandwidth).
        outr = outp.rearrange("(o p) c -> p o c", p=P)
        for ic in range(NI):
            isl = slice(ic * 512, (ic + 1) * 512)
            o_sb = [None] * HL
            og_stg = workp.tile([64, EC, 512], F32, name="og_stg", tag="ogstg")
            o_ps = [
                psum.tile([D + 1, 512], F32, tag="o", name=f"o_ps{h}", bufs=4)
                for h in range(HL)
            ]
            for jc in range(NJ):
                eb_sb = biasp.tile([P, 512], F32, name="eb_sb", tag="eb")
                # bias rides the ACT HWDGE queue (SP queue carries bulk loads)
                nc.scalar.dma_start(eb_sb, eb[jc * P : (jc + 1) * P, isl])
                for hp in range(EC):
                    # S^T[j, i]: shared pair lhsT, per-head zero-padded rhs
                    s_ps = psum.tile([P, 2, 512], F32, tag="s", name="s_ps")
                    for hh in range(2):
                        nc.tensor.matmul(
                            s_ps[:, hh, :],
                            _r(kT_sb[:, hp, jc * P : (jc + 1) * P]),
                            _r(qTp_sb[:, 2 * hp + hh, isl]),
                            start=True,
                            stop=True,
                        )
                    es_sb = esp.tile([P, 2, 512], F32, name="es_sb", tag="es")
                    nc.scalar.activation(es_sb, s_ps, EXP)
                    pt_sb = ptp.tile([P, 2, 512], F32R, name="pt_sb", tag="pt")
                    ebb = eb_sb[:, None, :].to_broadcast([P, 2, 512])
                    # all multiplies on DVE: concurrent GPSIMD elementwise
                    # work contends for SBUF ports and degrades DVE ~2.5x
                    nc.vector.tensor_tensor(pt_sb, es_sb, ebb, MUL)
                    for hh in range(2):
                        h = hp * 2 + hh
                        # O^T rows 0:64, softmax denominator row 64
                        nc.tensor.matmul(
                            o_ps[h],
                            _r(v_sb[:, jc, h, :]),
                            _r(pt_sb[:, hh, :]),
                            start=(jc == 0),
                            stop=(jc == NJ - 1),
                        )
            # drain accumulators to SBUF immediately to free PSUM banks
            for h in range(HL):
                osb = odp.tile([D + 1, 512], F32, name=f"o_sb{h}", tag="od")
                nc.scalar.copy(osb, o_ps[h])
                o_sb[h] = osb
            # normalize: oc = O * (1/denom) per head; odd heads staged then
            # relocated; gate applied afterwards in pair layout
            ocp_sb = workp.tile([P, EC, 512], F32, name="ocp_sb", tag="ocp")
            for h in range(HL):
                hp, hh = h // 2, h % 2
                rec_sb = rec_sbs[h % 2]
                recf_sb = workp.tile([P, 512], F32, name="recf_sb", tag="recf")
                bc_ps = psum.tile([P, 512], F32, tag="s", name="bc_ps")
                # approx recip over the whole accumulator (base-64 single-row
                # slices miscompute in the custom-DVE path); only row 64 - the
                # softmax denominators - is consumed
                nc.vector.reciprocal_approx_fast(
                    out=recf_sb[0:65, :], in_=o_sb[h]
                )
                nc.vector.tensor_copy(
                    out=rec_sb[64:65, :], in_=recf_sb[64:65, :]
                )
                # broadcast row 64 to all partitions: ones.T @ rec (the only
                # nonzero row of rec is the reciprocal) - stays in 128x128 mode
                nc.tensor.matmul(
                    bc_ps, _r(ones_sb), rec_sb, start=True, stop=True
                )
                oc_dst = (
                    ocp_sb[0:64, hp, :] if hh == 0 else og_stg[:, hp, :]
                )
                nc.vector.tensor_tensor(
                    oc_dst, bc_ps[0:64, :], o_sb[h][0:64, :], MUL
                )
            nc.sync.dma_start(ocp_sb[64:128, :, :], og_stg)
            for hp in range(EC):
                nc.vector.tensor_tensor(
                    og_sb[:, hp, isl], ocp_sb[:, hp, :], gp_sb[:, hp, isl], MUL
                )

            # ---- output projection for this ic (still 128x128) ----
            out_sb = odp.tile([P, 4, CQ], F32, name="out_sb", tag="outsb", bufs=2)
            for ip4 in range(4):
                ip = ic * 4 + ip4
                ps_o = psum.tile([P, 512], F32, tag="s", name="ps_o")
                for ec in range(EC):
                    nc.tensor.matmul(
                        ps_o,
                        _r(og_sb[:, ec, ip * P : (ip + 1) * P]),
                        _r(woT_sb[:, ec, :]),
                        start=(ec == 0),
                        stop=(ec == EC - 1),
                    )
                nc.vector.tensor_copy(out=out_sb[:, ip4, :], in_=ps_o)
            nc.sync.dma_start(outr[:, ic * 4 : (ic + 1) * 4, :], out_sb)


_CACHE = {}


def _get_nc():
    if "nc" not in _CACHE:
        nc = bacc.Bacc("TRN2", debug=False, enable_asserts=False)
        xT = nc.dram_tensor("xt_in", [CQ, Q], F32R, kind="ExternalInput").ap()
        eb = nc.dram_tensor("eb_in", [Q, QL], F32, kind="ExternalInput").ap()
        wT = nc.dram_tensor("wt_in", [CQ, 4 * EL], F32R, kind="ExternalInput").ap()
        woT = nc.dram_tensor("wot_in", [EL, CQ], F32R, kind="ExternalInput").ap()
        bg = nc.dram_tensor("bg_in", [EL, 1], F32, kind="ExternalInput").ap()
        ones_in = nc.dram_tensor("ones_in", [P, P], F32R, kind="ExternalInput").ap()
        ident_in = nc.dram_tensor("ident_in", [P, P], F32R, kind="ExternalInput").ap()
        zz_in = nc.dram_tensor("zz_in", [64, 2 * QL], F32R, kind="ExternalInput").ap()
        outp = nc.dram_tensor("out", [QL, CQ], F32, kind="ExternalOutput").ap()
        with tile.TileContext(nc) as tc:
            _emit(tc, xT, eb, wT, woT, bg, ones_in, zz_in, ident_in, outp)
        nc.compile()
        _CACHE["nc"] = nc
    return _CACHE["nc"]


LAST_RESULTS = None


def kernel(q_x, kv_x, bias, w_qkv, w_o, b_o, w_g, b_g):
    global LAST_RESULTS
    q_x = np.asarray(q_x, np.float32)
    bias = np.asarray(bias, np.float32)
    w_qkv = np.asarray(w_qkv, np.float32)
    w_o = np.asarray(w_o, np.float32)
    b_o = np.asarray(b_o, np.float32)
    w_g = np.asarray(w_g, np.float32)
    b_g = np.asarray(b_g, np.float32)

    ones = np.ones((P, P), np.float32)
    ident = np.eye(P, dtype=np.float32)
    zz = np.zeros((64, 2 * QL), np.float32)
    in_maps = []
    for core in range(8):
        b, qh, hq = core >> 2, (core >> 1) & 1, core & 1
        i0 = qh * QL
        esl = slice(hq * EL, (hq + 1) * EL)
        xTb = q_x[b].T  # [512, 2048]
        # roll keys so this core's queries are columns 0:QL
        xTp = np.concatenate([xTb[:, i0:], xTb[:, :i0]], axis=1)
        biasTb = bias[b, 0].T  # [keys, queries]
        ebp = np.exp(
            np.concatenate(
                [biasTb[i0:, i0 : i0 + QL], biasTb[:i0, i0 : i0 + QL]], axis=0
            )
        )
        wq = w_qkv[0:CQ][esl] * (1.0 / np.sqrt(D))
        wk = w_qkv[CQ : 2 * CQ][esl]
        wv = w_qkv[2 * CQ : 3 * CQ][esl]
        wg = w_g[esl]
        wTc = np.concatenate([wq.T, wk.T, wv.T, wg.T], axis=1)  # [512, 1024]
        woTc = w_o[:, esl].T  # [256, 512] pair-major rows
        bgc = (0.5 * b_g[esl]).reshape(EL, 1)
        in_maps.append(
            {
                "xt_in": np.ascontiguousarray(xTp, np.float32),
                "eb_in": np.ascontiguousarray(ebp, np.float32),
                "wt_in": np.ascontiguousarray(wTc, np.float32),
                "wot_in": np.ascontiguousarray(woTc, np.float32),
                "bg_in": np.ascontiguousarray(bgc, np.float32),
                "ones_in": ones,
                "ident_in": ident,
                "zz_in": zz,
            }
        )

    nc = _get_nc()
    res = run_bass_kernel_spmd(nc, in_maps, core_ids=list(range(8)))
    LAST_RESULTS = res

    out = np.zeros((B, Q, CQ), np.float32)
    for core in range(8):
        b, qh = core >> 2, (core >> 1) & 1
        i0 = qh * QL
        out[b, i0 : i0 + QL] += res.results[core]["out"]
    out += b_o
    return out



# revision 7
# speedup vs baseline: 1.3364x; 1.3364x over previous
"""Trainium2 Bass kernel for nn_Attention_73289321939579.

Gated attention block (AlphaFold-style):
  qkv = q_x @ w_qkv.T ; q /= sqrt(64)
  scores = q k^T + bias ; attn = softmax(scores, keys)
  o = (attn @ v) * sigmoid(q_x @ w_g.T + b_g)
  out = o @ w_o.T + b_o

Sharding over 8 cores: core = b*4 + qh*2 + hq
  b  = batch (2)            -> data parallel
  qh = query half (2x1024)  -> bias/q sliced, output row-sliced
  hq = head quad (2x4 heads)-> tensor parallel; partial outputs summed on host

Device layout (per core, contractions on the SBUF partition axis):
  xT   [512, 2048] bf16 = q_x[b].T, key-rolled so this core's queries are
        cols 0:1024
  eb   [2048, 1024] bf16 = exp(bias[b,0]).T (rows key-rolled, cols
        query-sliced); softmax computed as exp(qk) * exp(bias)
  wT   [512, 1024] bf16 = [wq.T/8 | wk.T | wv.T | wg.T] for 4 heads
  woT  [256, 512]  bf16 = w_o[:, heads].T (pair-major)
  bg   [256, 1]    f32  = 0.5*b_g[heads]  (gate via 0.5*tanh(0.5x+0.5bg)+0.5;
        Tanh shares the ACT "exp" table set -> no table swap)

Schedule (the previous 200us version ran the S->exp->mult->O chain nearly
serially and had ~15us serial normalize/out-proj lulls between query
blocks; ACT exp at ~1.04us/step is the real per-step floor):
  - all inputs bf16: halves DMA bytes + LDWEIGHTS + SBUF port traffic and
    turns the DVE multiply into 2x mode (~0.6us vs ~1.1us)
  - eb is loaded fully resident up front (scalar HWDGE queue) so no
    per-step DMA sits on the critical path
  - warmup burst cut to 16 matmuls (PE clock is fully ramped after ~3us
    of continuous activity)
  - attention runs as 4 passes (ic x hp) of 16 key-chunk steps; O-matmuls
    are emitted 2 steps behind S/exp/mult so the PE never waits on the
    exp pipeline; PSUM = 2x2-bank S slots + 2x1-bank O accumulators +
    2x1-bank out-proj/broadcast slots = exactly 8 banks
  - each pass's normalize + gate + out-projection is carried as a closure
    list and interleaved 2-ops-per-step into the NEXT pass, so the old
    inter-block lull disappears; denominator row moves ride SBUF->SBUF
    DMAs instead of the (saturated) ACT/DVE engines
  - normalize is pair-wise: both heads' softmax denominators are
    assembled into one tile, reciprocal'd together, and broadcast with a
    single bsel matmul (bsel row 64 -> partitions 0:64, row 65 -> 64:128)
All matmuls are plain 128x128 mode (mode switches drain the PE array).
"""

import sys

for _p in ("/opt/trn_rl_repo",):
    if _p not in sys.path:
        sys.path.insert(0, _p)

import math
from collections import deque
from contextlib import ExitStack

import ml_dtypes
import numpy as np

import concourse.bass as bass  # noqa: F401
import concourse.mybir as mybir
import concourse.tile as tile
from concourse import bacc
from concourse.bass_utils import run_bass_kernel_spmd

# ---- problem dims (hardcoded per contest contract) ----
B, Q, CQ = 2, 2048, 512
H, D = 8, 64
P = 128
QL = 1024          # queries per core
EL = 256           # e-dims per core (4 heads x 64)
HL = 4             # heads per core
CC = CQ // P       # 4 contraction chunks over channels
EC = EL // P       # 2 head-pairs
NJ = Q // P        # 16 key chunks
NI = QL // 512     # 2 query chunks of 512

F32 = mybir.dt.float32
F32R = mybir.dt.float32r
BF16 = mybir.dt.bfloat16
MUL = mybir.AluOpType.mult
ADD = mybir.AluOpType.add
EXP = mybir.ActivationFunctionType.Exp
TANH = mybir.ActivationFunctionType.Tanh

OFF_Q, OFF_K, OFF_V, OFF_G = 0, EL, 2 * EL, 3 * EL


def _r(ap):
    """float32r view for matmul operands (single-pass fp22 on the PE)."""
    return ap.bitcast(F32R)


def _emit(tc, xT, eb, wT, woT, bg, bsel, outp):
    nc = tc.nc

    with ExitStack() as ctx:
        const = ctx.enter_context(tc.tile_pool(name="const", bufs=1))
        esp = ctx.enter_context(tc.tile_pool(name="esp", bufs=3))
        ptp = ctx.enter_context(tc.tile_pool(name="ptp", bufs=4))
        workp = ctx.enter_context(tc.tile_pool(name="workp", bufs=2))
        psum = ctx.enter_context(tc.tile_pool(name="psum", bufs=2, space="PSUM"))

        # ---- constants built on-chip (no host DMA needed) ----
        ones_sb = const.tile([P, P], F32, name="ones_sb", tag="ones_sb")
        nc.vector.memset(ones_sb, 1.0)
        # bsel: row 64 -> output partitions 0:64, row 65 -> 64:128, so one
        # matmul broadcasts both heads' reciprocal rows to their O halves
        # (host-supplied: engine writes at partition base 65 are illegal)
        bsel_sb = const.tile([P, P], F32, name="bsel_sb", tag="bsel_sb")
        nc.sync.dma_start(bsel_sb, bsel)
        bg_sb = const.tile([P, EC], F32, name="bg_sb", tag="bg_sb")
        nc.sync.dma_start(bg_sb, bg.rearrange("(o p) u -> p (o u)", p=P))
        woT_sb = const.tile([P, EC, CQ], BF16, name="woT_sb", tag="woT_sb")
        nc.sync.dma_start(woT_sb, woT.rearrange("(o p) c -> p o c", p=P))
        # rec tiles: zero except rows 64:66 rewritten per pass
        rec_sbs = []
        for ri in range(2):
            rcb = const.tile([P, 512], F32, name=f"rec_sb{ri}", tag=f"rec_sb{ri}")
            nc.vector.memset(rcb, 0.0)
            rec_sbs.append(rcb)

        # ---- bulk inputs, chunked, in consumption order ----
        wT_sb = const.tile([P, CC, 4 * EL], BF16, name="wT_sb", tag="wT_sb")
        wTr = wT.rearrange("(o p) n -> p o n", p=P)
        for c in range(CC):
            nc.sync.dma_start(wT_sb[:, c, :], wTr[:, c, :])
        xT_sb = const.tile([P, CC, Q], BF16, name="xT_sb", tag="xT_sb")
        xTr = xT.rearrange("(o p) q -> p o q", p=P)
        for j4 in range(Q // 512):
            for c in range(CC):
                nc.sync.dma_start(
                    xT_sb[:, c, j4 * 512 : (j4 + 1) * 512],
                    xTr[:, c, j4 * 512 : (j4 + 1) * 512],
                )
        # exp(bias), fully resident: [key128, ic, jc, q] (scalar HWDGE queue
        # so it doesn't sit behind the xT stream on the SP queue)
        ebt_sb = const.tile([P, NI, NJ, 512], BF16, name="ebt_sb", tag="ebt_sb")
        for ic in range(NI):
            for jc in range(NJ):
                nc.scalar.dma_start(
                    ebt_sb[:, ic, jc, :],
                    eb[jc * P : (jc + 1) * P, ic * 512 : (ic + 1) * 512],
                )

        # ---- resident intermediates ----
        kT_sb = const.tile([P, EC, Q], BF16, name="kT_sb", tag="kT_sb")
        # per-head q, zero-padded in the other head's 64 partitions
        qTp_sb = const.tile([P, HL, QL], BF16, name="qTp_sb", tag="qTp_sb")
        nc.vector.memset(qTp_sb[64:128, 0::2, :], 0.0)
        nc.vector.memset(qTp_sb[0:64, 1::2, :], 0.0)
        gp_sb = const.tile([P, EC, QL], BF16, name="gp_sb", tag="gp_sb")
        og_sb = const.tile([P, EC, QL], BF16, name="og_sb", tag="og_sb")
        # V augmented with a ones column: [j, head, 65]
        v_sb = const.tile([P, NJ, HL, D + 1], BF16, name="v_sb", tag="v_sb")
        nc.vector.tensor_copy(
            out=v_sb[:, :, :, D],
            in_=ones_sb[:, 0:64].rearrange("p (a b) -> p a b", a=NJ),
        )

        # ---- phase 0: short PE warmup (clock ramps after ~3us active) ----
        warm_ps = psum.tile([P, 2, 512], F32, tag="s", name="warm_ps")
        for wi in range(12):
            nc.tensor.matmul(
                warm_ps[:, 0, 0:P],
                ones_sb,
                ones_sb,
                start=(wi == 0),
                stop=(wi == 11),
            )
        warm_sb = workp.tile([P, P], F32, name="warm_sb", tag="warm")
        nc.vector.tensor_copy(out=warm_sb[:, 0:P], in_=warm_ps[:, 0, 0:P])

        # ---- phase 1: projections (all 128x128, drains pinned to DVE) ----
        # kT pair-layout over all 2048 keys
        for j4 in range(Q // 512):
            for ec in range(EC):
                ps_k = psum.tile([P, 512], F32, tag="s", name="ps_k")
                for c in range(CC):
                    nc.tensor.matmul(
                        ps_k,
                        wT_sb[:, c, OFF_K + ec * P : OFF_K + (ec + 1) * P],
                        xT_sb[:, c, j4 * 512 : (j4 + 1) * 512],
                        start=(c == 0),
                        stop=(c == CC - 1),
                    )
                nc.vector.tensor_copy(
                    out=kT_sb[:, ec, j4 * 512 : (j4 + 1) * 512], in_=ps_k
                )
        # q pair-projected, split into the zero-padded per-head layout
        for ic in range(NI):
            for ec in range(EC):
                ps_q = psum.tile([P, 512], F32, tag="s", name="ps_q")
                for c in range(CC):
                    nc.tensor.matmul(
                        ps_q,
                        wT_sb[:, c, OFF_Q + ec * P : OFF_Q + (ec + 1) * P],
                        xT_sb[:, c, ic * 512 : (ic + 1) * 512],
                        start=(c == 0),
                        stop=(c == CC - 1),
                    )
                sl = slice(ic * 512, (ic + 1) * 512)
                nc.vector.tensor_copy(
                    out=qTp_sb[0:64, 2 * ec, sl], in_=ps_q[0:64, :]
                )
                nc.vector.tensor_copy(
                    out=qTp_sb[64:128, 2 * ec + 1, sl], in_=ps_q[64:128, :]
                )
        # v[j, e] (keys on partitions), scattered into the aug layout
        for jc in range(NJ):
            ps_v = psum.tile([P, 512], F32, tag="s", name="ps_v")
            for c in range(CC):
                nc.tensor.matmul(
                    ps_v[:, :EL],
                    xT_sb[:, c, jc * P : (jc + 1) * P],
                    wT_sb[:, c, OFF_V : OFF_V + EL],
                    start=(c == 0),
                    stop=(c == CC - 1),
                )
            nc.vector.tensor_copy(
                out=v_sb[:, jc, :, 0:D],
                in_=ps_v[:, :EL].rearrange("p (h d) -> p h d", h=HL),
            )
        # gate pairs: sigmoid(x wg^T + bg) = 0.5*tanh(0.5 x wg^T + 0.5 bg)+0.5
        for ec in range(EC):
            for ic in range(NI):
                ps_g = psum.tile([P, 512], F32, tag="s", name="ps_g")
                for c in range(CC):
                    nc.tensor.matmul(
                        ps_g,
                        wT_sb[:, c, OFF_G + ec * P : OFF_G + (ec + 1) * P],
                        xT_sb[:, c, ic * 512 : (ic + 1) * 512],
                        start=(c == 0),
                        stop=(c == CC - 1),
                    )
                nc.scalar.activation(
                    gp_sb[:, ec, ic * 512 : (ic + 1) * 512],
                    ps_g,
                    TANH,
                    bias=bg_sb[:, ec : ec + 1],
                    scale=0.5,
                )
        nc.vector.tensor_scalar(gp_sb, gp_sb, 0.5, 0.5, MUL, ADD)

        # ---- phase 2: attention, 4 passes of 16 pipelined steps ----
        outr = outp.rearrange("(o p) c -> p o c", p=P)

        def make_tail(ic, hp, o_ps, parity):
            """Closure list: normalize + gate for one finished pass."""
            isl = slice(ic * 512, (ic + 1) * 512)
            rec_sb = rec_sbs[parity]
            o_pair = workp.tile([P, 512], F32, name="o_pair", tag="opair")
            wk = workp.tile([P, 512], F32, name="wk", tag="wk")
            recf = workp.tile([P, 512], F32, name="recf", tag="recf")
            ocp = workp.tile([P, 512], BF16, name="ocp", tag="ocp")
            bc_ps = psum.tile([P, 512], F32, tag="op", name="bc_ps")
            ops = []
            # even head -> o_pair rows 0:64 (+denom in row 64 until moved)
            ops.append(lambda: nc.vector.tensor_copy(out=o_pair[0:65, :], in_=o_ps[0]))
            # odd head -> wk rows 0:64, its denom lands in wk[64]
            ops.append(lambda: nc.vector.tensor_copy(out=wk[0:65, :], in_=o_ps[1]))

            def _moves():
                # same SP queue -> the row read of o_pair[64] completes
                # before the block write overwrites it
                nc.sync.dma_start(wk[65:66, :], o_pair[64:65, :])
                nc.sync.dma_start(o_pair[64:128, :], wk[0:64, :])

            ops.append(_moves)
            ops.append(
                lambda: nc.vector.reciprocal_approx_fast(
                    out=recf[0:66, :], in_=wk[0:66, :]
                )
            )

            def _recrows():
                # recf[64] = 1/denom_odd -> rec row 65; recf[65] = 1/denom_even
                nc.sync.dma_start(rec_sb[65:66, :], recf[64:65, :])
                nc.sync.dma_start(rec_sb[64:65, :], recf[65:66, :])

            ops.append(_recrows)
            ops.append(
                lambda: nc.tensor.matmul(
                    bc_ps, bsel_sb, rec_sb, start=True, stop=True
                )
            )
            ops.append(lambda: nc.vector.tensor_tensor(ocp, bc_ps, o_pair, MUL))
            ops.append(
                lambda: nc.vector.tensor_tensor(
                    og_sb[:, hp, isl], ocp, gp_sb[:, hp, isl], MUL
                )
            )
            return ops

        def make_outproj(ic):
            """Closure list: out-projection + store for one query block."""
            ops = []
            for ip4 in range(4):
                ip = ic * 4 + ip4
                ps_o = psum.tile([P, 512], F32, tag="op", name="ps_o")
                out_sb = workp.tile([P, 512], F32, name="out_sb", tag="outsb")

                def _mm(ps_o=ps_o, ip=ip):
                    for ec in range(EC):
                        nc.tensor.matmul(
                            ps_o,
                            og_sb[:, ec, ip * P : (ip + 1) * P],
                            woT_sb[:, ec, :],
                            start=(ec == 0),
                            stop=(ec == EC - 1),
                        )

                def _st(ps_o=ps_o, out_sb=out_sb, ip=ip):
                    nc.vector.tensor_copy(out=out_sb, in_=ps_o)
                    nc.sync.dma_start(outr[:, ip, :], out_sb)

                ops.append(_mm)
                ops.append(_st)
            return ops

        tailq = deque()
        DEPTH = 2
        for pi, (ic, hp) in enumerate([(0, 0), (0, 1), (1, 0), (1, 1)]):
            isl = slice(ic * 512, (ic + 1) * 512)
            o_ps = [
                psum.tile([D + 1, 512], F32, tag="o", name=f"o_ps{hh}", bufs=2)
                for hh in range(2)
            ]
            pts = {}
            for jc in range(NJ):
                s_ps = psum.tile([P, 2, 512], F32, tag="s", name="s_ps")
                for hh in range(2):
                    nc.tensor.matmul(
                        s_ps[:, hh, :],
                        kT_sb[:, hp, jc * P : (jc + 1) * P],
                        qTp_sb[:, 2 * hp + hh, isl],
                        start=True,
                        stop=True,
                    )
                es = esp.tile([P, 2, 512], BF16, name="es", tag="es")
                nc.scalar.activation(es, s_ps, EXP)
                pt = ptp.tile([P, 2, 512], BF16, name="pt", tag="pt")
                ebb = ebt_sb[:, ic, jc, :][:, None, :].to_broadcast([P, 2, 512])
                nc.vector.tensor_tensor(pt, es, ebb, MUL)
                pts[jc] = pt
                # interleave carried tail work from the previous pass
                for _ in range(2):
                    if tailq:
                        tailq.popleft()()
                if jc >= DEPTH:
                    jd = jc - DEPTH
                    for hh in range(2):
                        nc.tensor.matmul(
                            o_ps[hh],
                            v_sb[:, jd, 2 * hp + hh, :],
                            pts[jd][:, hh, :],
                            start=(jd == 0),
                            stop=False,
                        )
                    del pts[jd]
            for jd in range(NJ - DEPTH, NJ):
                for hh in range(2):
                    nc.tensor.matmul(
                        o_ps[hh],
                        v_sb[:, jd, 2 * hp + hh, :],
                        pts[jd][:, hh, :],
                        start=False,
                        stop=(jd == NJ - 1),
                    )
                del pts[jd]
            tailq.extend(make_tail(ic, hp, o_ps, parity=pi % 2))
            if hp == 1:
                tailq.extend(make_outproj(ic))
        while tailq:
            tailq.popleft()()


_CACHE = {}


def _get_nc():
    if "nc" not in _CACHE:
        nc = bacc.Bacc("TRN2", debug=False, enable_asserts=False)
        xT = nc.dram_tensor("xt_in", [CQ, Q], BF16, kind="ExternalInput").ap()
        eb = nc.dram_tensor("eb_in", [Q, QL], BF16, kind="ExternalInput").ap()
        wT = nc.dram_tensor("wt_in", [CQ, 4 * EL], BF16, kind="ExternalInput").ap()
        woT = nc.dram_tensor("wot_in", [EL, CQ], BF16, kind="ExternalInput").ap()
        bg = nc.dram_tensor("bg_in", [EL, 1], F32, kind="ExternalInput").ap()
        bsel = nc.dram_tensor("bsel_in", [P, P], F32, kind="ExternalInput").ap()
        outp = nc.dram_tensor("out", [QL, CQ], F32, kind="ExternalOutput").ap()
        with tile.TileContext(nc) as tc:
            _emit(tc, xT, eb, wT, woT, bg, bsel, outp)
        nc.compile()
        _CACHE["nc"] = nc
    return _CACHE["nc"]


LAST_RESULTS = None
BF = ml_dtypes.bfloat16
_BSEL = np.zeros((P, P), np.float32)
_BSEL[64, 0:64] = 1.0
_BSEL[65, 64:128] = 1.0


def kernel(q_x, kv_x, bias, w_qkv, w_o, b_o, w_g, b_g):
    global LAST_RESULTS
    q_x = np.asarray(q_x, np.float32)
    bias = np.asarray(bias, np.float32)
    w_qkv = np.asarray(w_qkv, np.float32)
    w_o = np.asarray(w_o, np.float32)
    b_o = np.asarray(b_o, np.float32)
    w_g = np.asarray(w_g, np.float32)
    b_g = np.asarray(b_g, np.float32)

    in_maps = []
    for core in range(8):
        b, qh, hq = core >> 2, (core >> 1) & 1, core & 1
        i0 = qh * QL
        esl = slice(hq * EL, (hq + 1) * EL)
        xTb = q_x[b].T  # [512, 2048]
        # roll keys so this core's queries are columns 0:QL
        xTp = np.concatenate([xTb[:, i0:], xTb[:, :i0]], axis=1)
        biasTb = bias[b, 0].T  # [keys, queries]
        ebp = np.exp(
            np.concatenate(
                [biasTb[i0:, i0 : i0 + QL], biasTb[:i0, i0 : i0 + QL]], axis=0
            )
        )
        wq = w_qkv[0:CQ][esl] * (1.0 / np.sqrt(D))
        wk = w_qkv[CQ : 2 * CQ][esl]
        wv = w_qkv[2 * CQ : 3 * CQ][esl]
        wg = w_g[esl]
        wTc = np.concatenate([wq.T, wk.T, wv.T, wg.T], axis=1)  # [512, 1024]
        woTc = w_o[:, esl].T  # [256, 512] pair-major rows
        bgc = (0.5 * b_g[esl]).reshape(EL, 1)
        in_maps.append(
            {
                "xt_in": np.ascontiguousarray(xTp).astype(BF),
                "eb_in": np.ascontiguousarray(ebp).astype(BF),
                "wt_in": np.ascontiguousarray(wTc).astype(BF),
                "wot_in": np.ascontiguousarray(woTc).astype(BF),
                "bg_in": np.ascontiguousarray(bgc, np.float32),
                "bsel_in": _BSEL,
            }
        )

    nc = _get_nc()
    res = run_bass_kernel_spmd(nc, in_maps, core_ids=list(range(8)))
    LAST_RESULTS = res

    out = np.zeros((B, Q, CQ), np.float32)
    for core in range(8):
        b, qh = core >> 2, (core >> 1) & 1
        i0 = qh * QL
        out[b, i0 : i0 + QL] += res.results[core]["out"]
    out += b_o
    return out
